# revision 26
# baseline (speedup 1.0000x reference)
"""Multi-head causal self-attention on 8 TRN2 NeuronCores.

Problem: B=2, T=4096, D=512, H=8 heads (hd=64), fp32 in/out.

Sharding: core c in 0..7 handles batch b = c//4 and head pair g = c%4
(heads 2g, 2g+1 -> D-slice [128g, 128g+128)). Each core computes
    partial_out = concat_h( softmax(causal(Q_h K_h^T / 8)) V_h ) @ W_O[slice]
for its two heads; the host sums the 4 partials per batch and adds b_O.

Pipeline design (v5). ScalarE exp() is the hard floor: 144 score-slot
ACTIVATEs totalling ~156us busy at (N+352)cyc/1.2GHz. Everything else
is organised so the exp stream starts early and never starves:

  - Score slot = ONE key block x BOTH heads in one [128,1024] PSUM tile
    (head A bank 0, head B bank 1), one exp ACTIVATE per slot through a
    [128,2,n] strided AP -> 144 calls.  Global unit stream: scores run
    1 unit ahead of exp, PV 1 unit behind, across slice boundaries.
  - zfinish(s): the zaug PSUM banks are released FAST by a bf16 CAST of
    the Z rows + an f32 copy of the L row per head (~2.8us DVE), so the
    next slice's first PV barely waits.  Normalisation happens off the
    critical path ON THE GPSIMD DMA QUEUE (its mid-chain DVE waits must
    never stall the sync queue or the PE): L bounces through DRAM to
    [128,4] (where DVE reciprocal is cheap), 1/L bounces back to
    q-order and is broadcast across partitions with a 0-stride DRAM
    read; fused tensor_mul then builds a pre-normalised [128,512] bf16
    zpair (head B partition-shifted by a gpsimd DMA).  The O-projection
    is then a SINGLE 128-contraction matmul per q-tile (both heads
    summed by the PE) plus one CAST evac.  O-proj items are paced
    ~9 units past the boundary so their PE matmul never enqueues
    before zpair is ready (a stalled PE insert blocks later scores).
  - Tail (slice 7): post-norm variant skipping the broadcast chain:
    unnormalised stacked zpair feeds two CONCURRENT row-tiled O-proj
    matmuls per q-tile; evacs scale by 1/L ([128,4] spread) per output
    row, split across the now-idle ScalarE and the DVE.
  - DMA plan: per-chunk 2D x^T descriptors (batched 3D gathers run
    2-5x slower).  wq rides the Scalar-engine HWDGE queue (idle before
    the first ACTIVATE) so ACT_TABLE_LOAD lands early;
    ident/mask/biases/wk/x0-1/wv/wo/x4-7 + output tiles on the sync
    queue; x2-3 + bvrep + zfinish chains on the GpSimd SWDGE queue.
    Slice-0 projections are emitted before the stream (the PE is
    otherwise idle while DMAs land); projections for slice s+1 are
    spread one small item per stream iteration.
  - Masked scores use NEG=-640 (exp(-80)~1e-35): mathematically zero
    but safe for a future int16 fast-exp path.
"""

import numpy as np

import concourse.bass as bass
import concourse.mybir as mybir
from concourse.tile import TileContext
from concourse.bass_utils import run_bass_kernel_spmd

try:
    import ml_dtypes

    _BF16 = ml_dtypes.bfloat16
except ImportError:  # pragma: no cover
    _BF16 = None

F32 = mybir.dt.float32
BF16 = mybir.dt.bfloat16

B, T, D, H = 2, 4096, 512, 8
HD = D // H  # 64
SW = 512  # q-slice width
NS = T // SW  # 8 q-slices
NKC = D // 128  # 4 contraction chunks for the projections
NTT = T // 128  # 32 t-tiles / key blocks
NEG = -640.0  # masked score offset: exp((s+NEG)/8) <= exp(-74) ~ 0


def _split_waits(nc, max_waits=1):
    """The staged walrus rejects >1 semaphore wait per instruction; hoist
    extras onto same-engine NoOps inserted right before the instruction."""
    counter = 0
    for f in nc.m.functions:
        for blk in f.blocks:
            insts = blk.instructions
            out, changed = [], False
            for ins in insts:
                si = getattr(ins, "sync_info", None)
                waits = list(si.on_wait) if si is not None and si.on_wait else []
                if len(waits) > max_waits:
                    changed = True
                    for w in waits[:-max_waits]:
                        counter += 1
                        nop = mybir.InstNoOp(
                            name=f"I-wsplit-{counter}",
                            engine=ins.engine,
                            ins=[],
                            outs=[],
                        )
                        nop.sync_info = mybir.SyncInfo(on_wait=[w], on_update=[])
                        out.append(nop)
                    ins.sync_info = mybir.SyncInfo(
                        on_wait=waits[-max_waits:], on_update=list(si.on_update)
                    )
                out.append(ins)
            if changed:
                blk.instructions = out
    return counter


def _bcast_partitions(ap, n):
    """Replace the partition dim of an AP with an n-wide 0-stride dim."""
    return bass.AP(
        tensor=ap.tensor, offset=ap.offset, ap=[[0, n]] + list(ap.ap[1:])
    )


def build_nc():
    nc = bass.Bass("TRN2")

    xt = nc.dram_tensor("xt", [D, T], BF16, kind="ExternalInput")
    wq = nc.dram_tensor("wq", [D, 128], BF16, kind="ExternalInput")
    wk = nc.dram_tensor("wk", [D, 128], BF16, kind="ExternalInput")
    wv = nc.dram_tensor("wv", [D, 128], BF16, kind="ExternalInput")
    wo = nc.dram_tensor("wo", [128, D], BF16, kind="ExternalInput")
    bq = nc.dram_tensor("bq", [128, 1], F32, kind="ExternalInput")
    bk = nc.dram_tensor("bk", [128, 1], F32, kind="ExternalInput")
    bv = nc.dram_tensor("bv", [1, 128], F32, kind="ExternalInput")
    out = nc.dram_tensor("out", [T, D], BF16, kind="ExternalOutput")

    # maskneg[k, q'] = 0 where q' >= k else NEG  (S^T diagonal subtile mask)
    mask_np = np.where(
        np.arange(128)[None, :] >= np.arange(128)[:, None], 0.0, NEG
    ).astype(np.float32)
    ident_np = np.eye(128, dtype=np.float32)
    ident_dram = nc.inline_tensor(ident_np.astype(_BF16), name="identc")
    mask_dram = nc.inline_tensor(mask_np.astype(_BF16), name="maskc")

    with TileContext(nc) as tc:
        with (
            tc.tile_pool(name="singles", bufs=1) as singles,
            tc.tile_pool(name="ps", bufs=2, space="PSUM") as ps,
            tc.tile_pool(name="ex", bufs=2, space="PSUM") as ext,
            tc.tile_pool(name="zps", bufs=1, space="PSUM") as zps,
            tc.tile_pool(name="pt", bufs=6) as ptp,
            tc.tile_pool(name="rb", bufs=2) as rbp,
            tc.tile_pool(name="rr", bufs=4) as rrp,
            tc.tile_pool(name="zn", bufs=2) as znp,
            tc.tile_pool(name="outp", bufs=3) as outp,
            tc.tile_pool(name="drp", bufs=2, space="DRAM") as drp,
        ):
            # ---------- DMA schedule ----------
            # scalar HWDGE queue (idle until first ACTIVATE): wq only, so
            # the ACT_TABLE_LOAD that precedes the warm exp lands early.
            wq_sb = singles.tile([128, NKC, 128], BF16, tag="wq")
            nc.scalar.dma_start(
                out=wq_sb[:, :, :], in_=wq[:, :].rearrange("(c p) n -> p c n", p=128)
            )
            # sync queue: ident, mask, biases, wk, x0, wv, x1, x2, wo
            ident_sb = singles.tile([128, 128], BF16, tag="ident")
            mask_sb = singles.tile([128, 128], BF16, tag="mask")
            nc.sync.dma_start(out=ident_sb[:, :], in_=ident_dram[:, :])
            nc.sync.dma_start(out=mask_sb[:, :], in_=mask_dram[:, :])
            bq_sb = singles.tile([128, 1], F32, tag="bq")
            bk_sb = singles.tile([128, 1], F32, tag="bk")
            nc.sync.dma_start(out=bq_sb[:, :], in_=bq[:, :])
            nc.sync.dma_start(out=bk_sb[:, :], in_=bk[:, :])
            wk_sb = singles.tile([128, NKC, 128], BF16, tag="wk")
            nc.sync.dma_start(
                out=wk_sb[:, :, :], in_=wk[:, :].rearrange("(c p) n -> p c n", p=128)
            )
            # per-chunk 2D x^T descriptors (a batched 3D gather runs 2-5x
            # slower on the DMA engine); slices 0-1 on sync, 2+ on gpsimd
            xts = [
                singles.tile([128, NKC, SW], BF16, tag=f"xts{s}", name=f"xts{s}")
                for s in range(NS)
            ]

            def load_xt(eng, s):
                for c in range(NKC):
                    eng.dma_start(
                        out=xts[s][:, c, :],
                        in_=xt[c * 128 : (c + 1) * 128, s * SW : (s + 1) * SW],
                    )

            load_xt(nc.sync, 0)
            wv_sb = singles.tile([128, NKC, 128], BF16, tag="wv")
            nc.sync.dma_start(
                out=wv_sb[:, :, :], in_=wv[:, :].rearrange("(c p) n -> p c n", p=128)
            )
            load_xt(nc.sync, 1)
            wo_sb = singles.tile([128, D], BF16, tag="wo")
            nc.sync.dma_start(out=wo_sb[:, :], in_=wo[:, :])
            # gpsimd SWDGE queue: x2, x3, bvrep (the per-slice zfinish DMA
            # chains join this queue later -- their mid-chain DVE waits must
            # not block the sync queue, which carries the output tiles)
            load_xt(nc.gpsimd, 2)
            load_xt(nc.gpsimd, 3)
            bvrep_sb = singles.tile([128, 128], F32, tag="bvrep")
            nc.gpsimd.dma_start(out=bvrep_sb[:, :], in_=_bcast_partitions(bv[:, :], 128))
            for s in range(4, NS):
                load_xt(nc.sync, s)

            # preload the exp table set while DMAs stream in (the
            # ACT_TABLE_LOAD pseudo-inst lands before this in queue order)
            warm_sb = singles.tile([1, 1], BF16, tag="warm")
            nc.scalar.activation(
                out=warm_sb[:, :],
                in_=bq_sb[0:1, 0:1],
                func=mybir.ActivationFunctionType.Exp,
                scale=0.125,
            )

            qt_sb = [
                singles.tile([128, SW], BF16, tag=f"qt{s}", name=f"qt_sb{s}")
                for s in range(NS)
            ]
            kt_sb = [
                singles.tile([128, SW], BF16, tag=f"kt{s}", name=f"kt_sb{s}")
                for s in range(NS)
            ]
            # V_aug pair per key block: [128(t), 130]; cols 0:64 head A,
            # col 64 ones(A), cols 65:129 head B, col 129 ones(B)
            va_sb = [
                singles.tile([128, 2 * (HD + 1)], BF16, tag=f"va{t}", name=f"va_sb{t}")
                for t in range(NTT)
            ]
            for t in range(NTT):
                nc.vector.memset(va_sb[t][:, HD : HD + 1], 1.0)
                nc.vector.memset(va_sb[t][:, 2 * HD + 1 : 2 * HD + 2], 1.0)

            hrows = (slice(0, HD), slice(HD, 128))

            # ---------- emit helpers ----------
            def emit_proj_q(s, w_sb=None, b_sb=None, dst=None):
                w_sb = w_sb if w_sb is not None else wq_sb
                sg = ext.tile([128, SW], F32, tag="ex", name="ps_q")
                for c in range(NKC):
                    nc.tensor.matmul(
                        sg[:, :],
                        lhsT=w_sb[:, c, :],
                        rhs=xts[s][:, c, :],
                        start=(c == 0),
                        stop=(c == NKC - 1),
                        skip_group_check=True,
                    )
                nc.vector.tensor_scalar_add(
                    (dst if dst is not None else qt_sb[s])[:, :],
                    sg[:, :],
                    (b_sb if b_sb is not None else bq_sb)[:, :],
                )

            def emit_proj_k(s):
                emit_proj_q(s, w_sb=wk_sb, b_sb=bk_sb, dst=kt_sb[s])

            def emit_proj_v_tt(s, tt):
                # one ex tile (1 bank) per t-tile so the DVE evac of tile N
                # never reads a bank the PE is still writing (fatal)
                t = 4 * s + tt
                sg = ext.tile([128, SW], F32, tag="ex", name="ps_v")
                for c in range(NKC):
                    nc.tensor.matmul(
                        sg[:, 0:128],
                        lhsT=xts[s][:, c, tt * 128 : (tt + 1) * 128],
                        rhs=wv_sb[:, c, :],
                        start=(c == 0),
                        stop=(c == NKC - 1),
                        skip_group_check=True,
                    )
                # evac + b_V add in one op: dst [128,2,64] strided
                dst3 = va_sb[t][:, 0 : 2 * (HD + 1)].rearrange(
                    "p (a b) -> p a b", a=2
                )[:, :, 0:HD]
                src3 = sg[:, 0:128].rearrange("p (a b) -> p a b", a=2)
                bv3 = bvrep_sb[:, :].rearrange("p (a b) -> p a b", a=2)
                nc.vector.tensor_add(dst3, src3, bv3)

            def emit_scores(unit):
                s, kb, n, qlo = unit[:4]
                qs = s * SW
                diag = kb * 128 >= qs
                sg = ps.tile([128, 2 * SW], F32, tag="sg", name="ps_sg")
                unit[4] = sg
                for h in range(2):
                    off = h * SW
                    nc.tensor.matmul(
                        sg[:, off : off + n],
                        lhsT=kt_sb[kb // 4][
                            hrows[h], (kb % 4) * 128 : (kb % 4 + 1) * 128
                        ],
                        rhs=qt_sb[s][hrows[h], qlo - qs : qlo - qs + n],
                        start=True,
                        stop=not diag,
                        skip_group_check=True,
                        tile_position=(h * HD, 0),
                    )
                if diag:
                    for h in range(2):
                        nc.tensor.matmul(
                            sg[:, h * SW : h * SW + 128],
                            lhsT=ident_sb[:, :],
                            rhs=mask_sb[:, :],
                            start=False,
                            stop=True,
                            skip_group_check=True,
                        )

            def emit_exp(unit):
                s, kb, n, qlo, sg = unit[:5]
                pt = ptp.tile([128, 2 * SW], BF16, tag="pt", name="pt")
                in3 = sg[:, :].rearrange("p (a b) -> p a b", a=2)[:, :, 0:n]
                out3 = pt[:, 0 : 2 * n].rearrange("p (a b) -> p a b", a=2)
                nc.scalar.activation(
                    out=out3,
                    in_=in3,
                    func=mybir.ActivationFunctionType.Exp,
                    scale=0.125,
                )
                unit[4] = pt

            def emit_pv(unit, zaug):
                s, kb, n, qlo, pt = unit[:5]
                qs = s * SW
                nkb = 4 * (s + 1)
                for h in range(2):
                    nc.tensor.matmul(
                        zaug[h][0 : HD + 1, qlo - qs : SW],
                        lhsT=va_sb[kb][:, h * (HD + 1) : (h + 1) * (HD + 1)],
                        rhs=pt[:, h * n : h * n + n],
                        start=(kb == 0),
                        stop=(kb == nkb - 1),
                        skip_group_check=True,
                    )

            # ---------- zfinish: fast PSUM release + pre-normalised zpair ----
            # oproj work items: (zpair, qs, j, ready_at); deferred DVE ops
            # as (due_g, fn) -- a DVE op emitted while its DMA inputs are
            # still in flight blocks the whole in-order DVE queue, which
            # blocks ex-pool WAR evacs, which stalls the PE and starves exp.
            # So every chain-dependent DVE op is EMITTED only when its data
            # has certainly landed.
            oproj_work = []
            deferred = []

            def emit_zfinish(sp, zaug, g0=0):
                # 1) per-head: bf16 CAST of the Z rows + f32 copy of the PSUM
                #    L row; bank h is released after its two reads (~1.4us),
                #    so the next slice's first PV barely waits
                zraw, lrs = [], []
                for h in range(2):
                    zr = znp.tile(
                        [HD, SW], BF16, tag=f"zraw{h}", name=f"zraw{h}"
                    )
                    nc.vector.tensor_copy(zr[:, :], zaug[h][0:HD, :])
                    lr = rrp.tile([1, SW], F32, tag="rr", name="lrow")
                    nc.vector.tensor_copy(lr[0:1, :], zaug[h][HD : HD + 1, :])
                    zraw.append(zr)
                    lrs.append(lr)
                # 2) part 1 of the chain, all on the GPSIMD queue: spread L
                #    through DRAM into [128,4] (where DVE reciprocal is
                #    cheap); head-B partition shift
                lis = []
                for h in range(2):
                    rd = drp.tile([1, SW], F32, tag=f"rd{h}", name="rd")
                    nc.gpsimd.dma_start(out=rd[:, :], in_=lrs[h][0:1, :])
                    li = rrp.tile([128, SW // 128], F32, tag=f"li{h}", name="li")
                    nc.gpsimd.dma_start(
                        out=li[:, :],
                        in_=rd[0, :].rearrange("(a p) -> p a", p=128),
                    )
                    lis.append(li)
                zpair = znp.tile([128, SW], BF16, tag="zp", name="zpair")
                zsh = znp.tile([128, SW], BF16, tag="zs", name="zsh")
                nc.gpsimd.dma_start(out=zsh[HD:128, :], in_=zraw[1][:, :])
                rbc = rbp.tile([128, SW], F32, tag="rb", name="rbc")
                st = {"lis": lis, "zraw": zraw, "zsh": zsh, "rbc": rbc,
                      "zpair": zpair}

                def part2(st=st):
                    # recips (their li spreads landed ~4 units ago) + the
                    # 1/L -> q-order -> partition-broadcast bounce
                    for h in range(2):
                        ri = rrp.tile(
                            [128, SW // 128], F32, tag=f"ri{h}", name="ri"
                        )
                        nc.vector.reciprocal(ri[:, :], st["lis"][h][:, :])
                        rd2 = drp.tile([1, SW], F32, tag=f"rd2{h}", name="rd2")
                        nc.gpsimd.dma_start(
                            out=rd2[0, :].rearrange("(a p) -> p a", p=128),
                            in_=ri[:, :],
                        )
                        nc.gpsimd.dma_start(
                            out=st["rbc"][h * HD : (h + 1) * HD, :],
                            in_=_bcast_partitions(rd2[:, :], HD),
                        )

                def part3(st=st):
                    # fused normalise into zpair (rbc landed ~2 units ago)
                    nc.vector.tensor_mul(
                        st["zpair"][0:HD, :], st["zraw"][0][:, :],
                        st["rbc"][0:HD, :],
                    )
                    nc.vector.tensor_mul(
                        st["zpair"][HD:128, :], st["zsh"][HD:128, :],
                        st["rbc"][HD:128, :],
                    )

                deferred.append((g0 + 8, part2))
                deferred.append((g0 + 12, part3))
                for j in range(4):
                    oproj_work.append((zpair, sp * SW, j, g0 + 14 + 2 * j))

            def emit_oproj_qtile(item, scalar_evac=False):
                zpair, qs_t, j, _ = item
                op = ext.tile([128, SW], F32, tag="ex", name="ps_o")
                jq = slice(j * 128, (j + 1) * 128)
                # both heads in ONE 128-contraction matmul (wo_sb rows are
                # the two heads' W_O rows stacked; zpair is pre-normalised)
                nc.tensor.matmul(
                    op[:, :],
                    lhsT=zpair[:, jq],
                    rhs=wo_sb[:, :],
                    start=True,
                    stop=True,
                    skip_group_check=True,
                )
                o_sb = outp.tile([128, D], BF16, tag="ot", name="o_sb")
                if scalar_evac:
                    nc.scalar.copy(o_sb[:, :], op[:, :])
                else:
                    nc.vector.tensor_copy(o_sb[:, :], op[:, :])
                r0 = qs_t + j * 128
                nc.sync.dma_start(out=out[r0 : r0 + 128, :], in_=o_sb[:, :])

            # HAM warm-up bridging preamble -> first projection
            warm_ps = ext.tile([128, SW], F32, tag="ex", name="ps_warm")
            for _ in range(20):
                nc.tensor.matmul(
                    warm_ps[:, 0:128],
                    lhsT=ident_sb[:, :],
                    rhs=mask_sb[:, :],
                    start=True,
                    stop=True,
                    skip_group_check=True,
                )

            # ---------- global unit stream ----------
            stream = []
            first_of_slice = {}
            for s in range(NS):
                qs = s * SW
                first_of_slice[s] = len(stream)
                for kb in range(4 * (s + 1)):
                    qlo = max(qs, kb * 128)
                    stream.append([s, kb, qs + SW - qlo, qlo, None])
            G = len(stream)

            def proj_items(sn):
                return [
                    lambda sn=sn: emit_proj_q(sn),
                    lambda sn=sn: emit_proj_k(sn),
                    lambda sn=sn: emit_proj_v_tt(sn, 0),
                    lambda sn=sn: emit_proj_v_tt(sn, 1),
                    lambda sn=sn: emit_proj_v_tt(sn, 2),
                    lambda sn=sn: emit_proj_v_tt(sn, 3),
                ]

            pending_proj = []  # for the upcoming slice

            def inserts(s, i, g, L):
                # 1) projection items for slice s+1 (hard deadline: all
                #    emitted before slice s+1 begins); 2/iter for short
                #    slices, else 1/iter starting at i=1
                budget = 2 if L <= 4 else 1
                did = 0
                while pending_proj and did < budget:
                    left = L - i
                    if len(pending_proj) >= left or i >= 1:
                        pending_proj.pop(0)()
                        did += 1
                    else:
                        break
                # 2) O-proj q-tiles when pacing allows and no proj emitted
                if did == 0 and oproj_work and g >= oproj_work[0][3]:
                    emit_oproj_qtile(oproj_work.pop(0))

            # prefill: slice-0 Q/K then the first scores (so exp starts
            # ASAP), then slice-0 V while exp(u0) runs
            emit_proj_q(0)
            emit_proj_k(0)
            emit_scores(stream[0])
            for tt in range(4):
                emit_proj_v_tt(0, tt)

            zaug = None
            prev_zaug = None
            for g in range(G):
                s, kb = stream[g][0], stream[g][1]
                L = 4 * (s + 1)
                i = g - first_of_slice[s]
                if i == 0 and s + 1 < NS:
                    pending_proj.extend(proj_items(s + 1))
                emit_exp(stream[g])
                if g + 1 < G:
                    emit_scores(stream[g + 1])
                if g >= 1:
                    ps_, pkb = stream[g - 1][0], stream[g - 1][1]
                    if pkb == 0:
                        # stream[g-1] opens slice ps_: finish the previous
                        # slice's accumulators, then claim fresh ones
                        if prev_zaug is not None:
                            emit_zfinish(ps_ - 1, prev_zaug, g0=g)
                        zaug = [
                            zps.tile([HD + 1, SW], F32, tag="za", name="zauga"),
                            zps.tile([HD + 1, SW], F32, tag="zb", name="zaugb"),
                        ]
                        prev_zaug = zaug
                    emit_pv(stream[g - 1], zaug)
                if deferred:
                    due = [f for dg, f in deferred if dg <= g]
                    deferred[:] = [(dg, f) for dg, f in deferred if dg > g]
                    for f in due:
                        f()
                inserts(s, i, g, L)
            emit_pv(stream[G - 1], zaug)
            for _, f in deferred:
                f()
            deferred.clear()

            # ---------- tail: slice 7 ----------
            # post-norm variant: skips the 1/L broadcast chain entirely.
            # Unnormalised zpair (stacked) feeds two CONCURRENT row-tiled
            # O-proj matmuls per q-tile into one score-pool tile's two
            # banks; the evacs scale by 1/L per output row, split across
            # the now-idle ScalarE and the DVE.
            zpair = znp.tile([128, SW], BF16, tag="zp", name="zpair_t")
            znb = znp.tile([HD, SW], BF16, tag=f"zraw1", name="znb_t")
            nc.vector.tensor_copy(zpair[0:HD, :], zaug[0][0:HD, :])
            nc.vector.tensor_copy(znb[:, :], zaug[1][0:HD, :])
            nc.gpsimd.dma_start(out=zpair[HD:128, :], in_=znb[:, :])
            ris = []
            for h in range(2):
                lr = rrp.tile([1, SW], F32, tag="rr", name="lrow_t")
                nc.vector.tensor_copy(lr[0:1, :], zaug[h][HD : HD + 1, :])
                rd = drp.tile([1, SW], F32, tag=f"rd{h}", name="rd_t")
                nc.sync.dma_start(out=rd[:, :], in_=lr[0:1, :])
                li = rrp.tile([128, SW // 128], F32, tag=f"li{h}", name="li_t")
                nc.sync.dma_start(
                    out=li[:, :], in_=rd[0, :].rearrange("(a p) -> p a", p=128)
                )
                ri = rrp.tile([128, SW // 128], F32, tag=f"ri{h}", name="ri_t")
                nc.vector.reciprocal(ri[:, :], li[:, :])
                ris.append(ri)
            for j in range(4):
                jq = slice(j * 128, (j + 1) * 128)
                op = ps.tile([128, 2 * SW], F32, tag="sg", name="ps_ot")
                nc.tensor.matmul(
                    op[:, 0:SW],
                    lhsT=zpair[0:HD, jq],
                    rhs=wo_sb[0:HD, :],
                    start=True,
                    stop=True,
                    skip_group_check=True,
                    tile_position=(0, 0),
                )
                nc.tensor.matmul(
                    op[:, SW : 2 * SW],
                    lhsT=zpair[HD:128, jq],
                    rhs=wo_sb[HD:128, :],
                    start=True,
                    stop=True,
                    skip_group_check=True,
                    tile_position=(HD, 0),
                )
                o_tmp = outp.tile([128, D], F32, tag="otmp", name="o_tmp")
                nc.scalar.mul(o_tmp[:, :], op[:, SW : 2 * SW], ris[1][:, j : j + 1])
                o_sb = outp.tile([128, D], BF16, tag="ot", name="o_sbt")
                nc.vector.scalar_tensor_tensor(
                    o_sb[:, :],
                    op[:, 0:SW],
                    ris[0][:, j : j + 1],
                    o_tmp[:, :],
                    op0=mybir.AluOpType.mult,
                    op1=mybir.AluOpType.add,
                )
                r0 = (NS - 1) * SW + j * 128
                nc.sync.dma_start(out=out[r0 : r0 + 128, :], in_=o_sb[:, :])
            # drain any remaining O-proj backlog (slice 6)
            k = 0
            while oproj_work:
                emit_oproj_qtile(oproj_work.pop(0), scalar_evac=(k % 2 == 0))
                k += 1

    _split_waits(nc)
    return nc


_NC_CACHE = {}


def _get_nc():
    if "nc" not in _NC_CACHE:
        _NC_CACHE["nc"] = build_nc()
    return _NC_CACHE["nc"]


def make_in_maps(combined_embed, W_K, b_K, W_Q, b_Q, W_V, b_V, W_O, b_O):
    f32 = np.float32
    in_maps = []
    for c in range(8):
        b = c // 4
        g = c % 4
        sl = slice(g * 128, (g + 1) * 128)
        xt = np.ascontiguousarray(np.asarray(combined_embed[b], f32).T)
        in_maps.append(
            {
                "xt": xt.astype(_BF16),
                "wq": np.ascontiguousarray(np.asarray(W_Q, f32)[:, sl]).astype(_BF16),
                "wk": np.ascontiguousarray(np.asarray(W_K, f32)[:, sl]).astype(_BF16),
                "wv": np.ascontiguousarray(np.asarray(W_V, f32)[:, sl]).astype(_BF16),
                "wo": np.ascontiguousarray(np.asarray(W_O, f32)[sl, :]).astype(_BF16),
                "bq": np.asarray(b_Q, f32)[sl].reshape(128, 1).copy(),
                "bk": np.asarray(b_K, f32)[sl].reshape(128, 1).copy(),
                "bv": np.asarray(b_V, f32)[sl].reshape(1, 128).copy(),
            }
        )
    return in_maps


def run_cores(in_maps, **kwargs):
    nc = _get_nc()
    return run_bass_kernel_spmd(nc, in_maps, core_ids=list(range(8)), **kwargs)


def kernel(
    combined_embed, W_K, b_K, W_Q, b_Q, W_V, b_V, W_O, b_O
):  # full inputs -> full output
    in_maps = make_in_maps(
        combined_embed, W_K, b_K, W_Q, b_Q, W_V, b_V, W_O, b_O
    )
    res = run_cores(in_maps)
    out = np.zeros((B, T, D), np.float32)
    for c in range(8):
        out[c // 4] += np.asarray(res.results[c]["out"], np.float32)
    out += np.asarray(b_O, np.float32)[None, None, :]
    return out


# revision 31
# speedup vs baseline: 1.2876x; 1.2876x over previous
"""Multi-head causal self-attention on 8 TRN2 NeuronCores.

Problem: B=2, T=4096, D=512, H=8 heads (hd=64), fp32 in/out.

Sharding: core c in 0..7 handles batch b = c//4 and head pair g = c%4
(heads 2g, 2g+1 -> D-slice [128g, 128g+128)). Each core computes
    partial_out = concat_h( softmax(causal(Q_h K_h^T / 8)) V_h ) @ W_O[slice]
for its two heads; the host sums the 4 partials per batch and adds b_O.

Pipeline design (v5). ScalarE exp() is the hard floor: 144 score-slot
ACTIVATEs totalling ~156us busy at (N+352)cyc/1.2GHz. Everything else
is organised so the exp stream starts early and never starves:

  - Score slot = ONE key block x BOTH heads in one [128,1024] PSUM tile
    (head A bank 0, head B bank 1), one exp ACTIVATE per slot through a
    [128,2,n] strided AP -> 144 calls.  Global unit stream: scores run
    1 unit ahead of exp, PV 1 unit behind, across slice boundaries.
  - zfinish(s): the zaug PSUM banks are released FAST by a bf16 CAST of
    the Z rows + an f32 copy of the L row per head (~2.8us DVE), so the
    next slice's first PV barely waits.  Normalisation happens off the
    critical path ON THE GPSIMD DMA QUEUE (its mid-chain DVE waits must
    never stall the sync queue or the PE): L bounces through DRAM to
    [128,4] (where DVE reciprocal is cheap), 1/L bounces back to
    q-order and is broadcast across partitions with a 0-stride DRAM
    read; fused tensor_mul then builds a pre-normalised [128,512] bf16
    zpair (head B partition-shifted by a gpsimd DMA).  The O-projection
    is then a SINGLE 128-contraction matmul per q-tile (both heads
    summed by the PE) plus one CAST evac.  O-proj items are paced
    ~9 units past the boundary so their PE matmul never enqueues
    before zpair is ready (a stalled PE insert blocks later scores).
  - Tail (slice 7): post-norm variant skipping the broadcast chain:
    unnormalised stacked zpair feeds two CONCURRENT row-tiled O-proj
    matmuls per q-tile; evacs scale by 1/L ([128,4] spread) per output
    row, split across the now-idle ScalarE and the DVE.
  - DMA plan: per-chunk 2D x^T descriptors (batched 3D gathers run
    2-5x slower).  wq rides the Scalar-engine HWDGE queue (idle before
    the first ACTIVATE) so ACT_TABLE_LOAD lands early;
    ident/mask/biases/wk/x0-1/wv/wo/x4-7 + output tiles on the sync
    queue; x2-3 + bvrep + zfinish chains on the GpSimd SWDGE queue.
    Slice-0 projections are emitted before the stream (the PE is
    otherwise idle while DMAs land); projections for slice s+1 are
    spread one small item per stream iteration.
  - Masked scores use NEG=-640 (exp(-80)~1e-35): mathematically zero
    but safe for a future int16 fast-exp path.
"""

import numpy as np

import concourse.bass as bass
import concourse.mybir as mybir
from concourse.tile import TileContext
from concourse.bass_utils import run_bass_kernel_spmd

try:
    import ml_dtypes

    _BF16 = ml_dtypes.bfloat16
except ImportError:  # pragma: no cover
    _BF16 = None

F32 = mybir.dt.float32
BF16 = mybir.dt.bfloat16

B, T, D, H = 2, 4096, 512, 8
HD = D // H  # 64
SW = 512  # q-slice width
NS = T // SW  # 8 q-slices
NKC = D // 128  # 4 contraction chunks for the projections
NTT = T // 128  # 32 t-tiles / key blocks
NEG = -640.0  # masked score offset: exp((s+NEG)/8) <= exp(-74) ~ 0


def _split_waits(nc, max_waits=1):
    """The staged walrus rejects >1 semaphore wait per instruction; hoist
    extras onto same-engine NoOps inserted right before the instruction."""
    counter = 0
    for f in nc.m.functions:
        for blk in f.blocks:
            insts = blk.instructions
            out, changed = [], False
            for ins in insts:
                si = getattr(ins, "sync_info", None)
                waits = list(si.on_wait) if si is not None and si.on_wait else []
                if len(waits) > max_waits:
                    changed = True
                    for w in waits[:-max_waits]:
                        counter += 1
                        nop = mybir.InstNoOp(
                            name=f"I-wsplit-{counter}",
                            engine=ins.engine,
                            ins=[],
                            outs=[],
                        )
                        nop.sync_info = mybir.SyncInfo(on_wait=[w], on_update=[])
                        out.append(nop)
                    ins.sync_info = mybir.SyncInfo(
                        on_wait=waits[-max_waits:], on_update=list(si.on_update)
                    )
                out.append(ins)
            if changed:
                blk.instructions = out
    return counter


def _bcast_partitions(ap, n):
    """Replace the partition dim of an AP with an n-wide 0-stride dim."""
    return bass.AP(
        tensor=ap.tensor, offset=ap.offset, ap=[[0, n]] + list(ap.ap[1:])
    )


def build_nc():
    nc = bass.Bass("TRN2")

    xt = nc.dram_tensor("xt", [D, T], BF16, kind="ExternalInput")
    wq = nc.dram_tensor("wq", [D, 128], BF16, kind="ExternalInput")
    wk = nc.dram_tensor("wk", [D, 128], BF16, kind="ExternalInput")
    wv = nc.dram_tensor("wv", [D, 128], BF16, kind="ExternalInput")
    wo = nc.dram_tensor("wo", [128, D], BF16, kind="ExternalInput")
    bq = nc.dram_tensor("bq", [128, 1], F32, kind="ExternalInput")
    bk = nc.dram_tensor("bk", [128, 1], F32, kind="ExternalInput")
    bv = nc.dram_tensor("bv", [1, 128], F32, kind="ExternalInput")
    out = nc.dram_tensor("out", [T, D], BF16, kind="ExternalOutput")

    # maskneg[k, q'] = 0 where q' >= k else NEG  (S^T diagonal subtile mask)
    mask_np = np.where(
        np.arange(128)[None, :] >= np.arange(128)[:, None], 0.0, NEG
    ).astype(np.float32)
    ident_np = np.eye(128, dtype=np.float32)
    ident_dram = nc.inline_tensor(ident_np.astype(_BF16), name="identc")
    mask_dram = nc.inline_tensor(mask_np.astype(_BF16), name="maskc")

    with TileContext(nc) as tc:
        with (
            tc.tile_pool(name="singles", bufs=1) as singles,
            tc.tile_pool(name="ps", bufs=2, space="PSUM") as ps,
            tc.tile_pool(name="ex", bufs=2, space="PSUM") as ext,
            tc.tile_pool(name="zps", bufs=1, space="PSUM") as zps,
            tc.tile_pool(name="pt", bufs=6) as ptp,
            tc.tile_pool(name="rb", bufs=2) as rbp,
            tc.tile_pool(name="rr", bufs=4) as rrp,
            tc.tile_pool(name="zn", bufs=2) as znp,
            tc.tile_pool(name="outp", bufs=3) as outp,
            tc.tile_pool(name="drp", bufs=2, space="DRAM") as drp,
        ):
            # ---------- DMA schedule ----------
            # scalar HWDGE queue (idle until first ACTIVATE): wq only, so
            # the ACT_TABLE_LOAD that precedes the warm exp lands early.
            wq_sb = singles.tile([128, NKC, 128], BF16, tag="wq")
            nc.scalar.dma_start(
                out=wq_sb[:, :, :], in_=wq[:, :].rearrange("(c p) n -> p c n", p=128)
            )
            # sync queue: ident, mask, biases, wk, x0, wv, x1, x2, wo
            ident_sb = singles.tile([128, 128], BF16, tag="ident")
            mask_sb = singles.tile([128, 128], BF16, tag="mask")
            nc.sync.dma_start(out=ident_sb[:, :], in_=ident_dram[:, :])
            nc.sync.dma_start(out=mask_sb[:, :], in_=mask_dram[:, :])
            # per-chunk 2D x^T descriptors (a batched 3D gather runs 2-5x
            # slower on the DMA engine); x0 FIRST -- the Q projection (and
            # with it the whole exp stream) starts the moment x0 lands
            xts = [
                singles.tile([128, NKC, SW], BF16, tag=f"xts{s}", name=f"xts{s}")
                for s in range(NS)
            ]

            def load_xt(eng, s):
                for c in range(NKC):
                    eng.dma_start(
                        out=xts[s][:, c, :],
                        in_=xt[c * 128 : (c + 1) * 128, s * SW : (s + 1) * SW],
                    )

            load_xt(nc.sync, 0)
            wk_sb = singles.tile([128, NKC, 128], BF16, tag="wk")
            nc.sync.dma_start(
                out=wk_sb[:, :, :], in_=wk[:, :].rearrange("(c p) n -> p c n", p=128)
            )
            bq_sb = singles.tile([128, 1], F32, tag="bq")
            bk_sb = singles.tile([128, 1], F32, tag="bk")
            nc.sync.dma_start(out=bq_sb[:, :], in_=bq[:, :])
            nc.sync.dma_start(out=bk_sb[:, :], in_=bk[:, :])
            wv_sb = singles.tile([128, NKC, 128], BF16, tag="wv")
            nc.sync.dma_start(
                out=wv_sb[:, :, :], in_=wv[:, :].rearrange("(c p) n -> p c n", p=128)
            )
            load_xt(nc.sync, 1)
            wo_sb = singles.tile([128, D], BF16, tag="wo")
            nc.sync.dma_start(out=wo_sb[:, :], in_=wo[:, :])
            # gpsimd SWDGE queue: x2, x3, bvrep (the per-slice zfinish DMA
            # chains join this queue later -- their mid-chain DVE waits must
            # not block the sync queue, which carries the output tiles)
            load_xt(nc.gpsimd, 2)
            load_xt(nc.gpsimd, 3)
            bvrep_sb = singles.tile([128, 128], F32, tag="bvrep")
            nc.gpsimd.dma_start(out=bvrep_sb[:, :], in_=_bcast_partitions(bv[:, :], 128))
            for s in range(4, NS):
                load_xt(nc.sync, s)

            # preload the exp table set while DMAs stream in (the
            # ACT_TABLE_LOAD pseudo-inst lands before this in queue order)
            warm_sb = singles.tile([1, 1], BF16, tag="warm")
            nc.scalar.activation(
                out=warm_sb[:, :],
                in_=bq_sb[0:1, 0:1],
                func=mybir.ActivationFunctionType.Exp,
                scale=0.125,
            )

            qt_sb = [
                singles.tile([128, SW], BF16, tag=f"qt{s}", name=f"qt_sb{s}")
                for s in range(NS)
            ]
            kt_sb = [
                singles.tile([128, SW], BF16, tag=f"kt{s}", name=f"kt_sb{s}")
                for s in range(NS)
            ]
            # V_aug pair per key block: [128(t), 130]; cols 0:64 head A,
            # col 64 ones(A), cols 65:129 head B, col 129 ones(B)
            va_sb = [
                singles.tile([128, 2 * (HD + 1)], BF16, tag=f"va{t}", name=f"va_sb{t}")
                for t in range(NTT)
            ]
            for t in range(NTT):
                nc.vector.memset(va_sb[t][:, HD : HD + 1], 1.0)
                nc.vector.memset(va_sb[t][:, 2 * HD + 1 : 2 * HD + 2], 1.0)

            hrows = (slice(0, HD), slice(HD, 128))

            # ---------- emit helpers ----------
            def emit_proj_q(s, w_sb=None, b_sb=None, dst=None):
                w_sb = w_sb if w_sb is not None else wq_sb
                sg = ext.tile([128, SW], F32, tag="ex", name="ps_q")
                for c in range(NKC):
                    nc.tensor.matmul(
                        sg[:, :],
                        lhsT=w_sb[:, c, :],
                        rhs=xts[s][:, c, :],
                        start=(c == 0),
                        stop=(c == NKC - 1),
                        skip_group_check=True,
                    )
                nc.vector.tensor_scalar_add(
                    (dst if dst is not None else qt_sb[s])[:, :],
                    sg[:, :],
                    (b_sb if b_sb is not None else bq_sb)[:, :],
                )

            def emit_proj_k(s):
                emit_proj_q(s, w_sb=wk_sb, b_sb=bk_sb, dst=kt_sb[s])

            def emit_proj_v_tt(s, tt):
                # one ex tile (1 bank) per t-tile so the DVE evac of tile N
                # never reads a bank the PE is still writing (fatal)
                t = 4 * s + tt
                sg = ext.tile([128, SW], F32, tag="ex", name="ps_v")
                for c in range(NKC):
                    nc.tensor.matmul(
                        sg[:, 0:128],
                        lhsT=xts[s][:, c, tt * 128 : (tt + 1) * 128],
                        rhs=wv_sb[:, c, :],
                        start=(c == 0),
                        stop=(c == NKC - 1),
                        skip_group_check=True,
                    )
                # evac + b_V add in one op: dst [128,2,64] strided
                dst3 = va_sb[t][:, 0 : 2 * (HD + 1)].rearrange(
                    "p (a b) -> p a b", a=2
                )[:, :, 0:HD]
                src3 = sg[:, 0:128].rearrange("p (a b) -> p a b", a=2)
                bv3 = bvrep_sb[:, :].rearrange("p (a b) -> p a b", a=2)
                nc.vector.tensor_add(dst3, src3, bv3)

            def emit_scores(unit):
                s, kb, n, qlo = unit[:4]
                qs = s * SW
                diag = kb * 128 >= qs
                sg = ps.tile([128, 2 * SW], F32, tag="sg", name="ps_sg")
                unit[4] = sg
                for h in range(2):
                    off = h * SW
                    nc.tensor.matmul(
                        sg[:, off : off + n],
                        lhsT=kt_sb[kb // 4][
                            hrows[h], (kb % 4) * 128 : (kb % 4 + 1) * 128
                        ],
                        rhs=qt_sb[s][hrows[h], qlo - qs : qlo - qs + n],
                        start=True,
                        stop=not diag,
                        skip_group_check=True,
                        tile_position=(h * HD, 0),
                    )
                if diag:
                    for h in range(2):
                        nc.tensor.matmul(
                            sg[:, h * SW : h * SW + 128],
                            lhsT=ident_sb[:, :],
                            rhs=mask_sb[:, :],
                            start=False,
                            stop=True,
                            skip_group_check=True,
                        )

            def emit_exp(unit):
                s, kb, n, qlo, sg = unit[:5]
                pt = ptp.tile([128, 2 * SW], BF16, tag="pt", name="pt")
                in3 = sg[:, :].rearrange("p (a b) -> p a b", a=2)[:, :, 0:n]
                out3 = pt[:, 0 : 2 * n].rearrange("p (a b) -> p a b", a=2)
                nc.scalar.activation(
                    out=out3,
                    in_=in3,
                    func=mybir.ActivationFunctionType.Exp,
                    scale=0.125,
                )
                unit[4] = pt

            def emit_pv(unit, zaug):
                s, kb, n, qlo, pt = unit[:5]
                qs = s * SW
                nkb = 4 * (s + 1)
                for h in range(2):
                    nc.tensor.matmul(
                        zaug[h][0 : HD + 1, qlo - qs : SW],
                        lhsT=va_sb[kb][:, h * (HD + 1) : (h + 1) * (HD + 1)],
                        rhs=pt[:, h * n : h * n + n],
                        start=(kb == 0),
                        stop=(kb == nkb - 1),
                        skip_group_check=True,
                    )

            # ---------- zfinish: fast PSUM release + pre-normalised zpair ----
            # oproj work items: (zpair, qs, j, ready_at); deferred DVE ops
            # as (due_g, fn) -- a DVE op emitted while its DMA inputs are
            # still in flight blocks the whole in-order DVE queue, which
            # blocks ex-pool WAR evacs, which stalls the PE and starves exp.
            # So every chain-dependent DVE op is EMITTED only when its data
            # has certainly landed.
            oproj_work = []
            deferred = []

            def emit_zfinish(sp, zaug, g0=0):
                # 1) per-head: bf16 CAST of the Z rows (head A straight into
                #    zpair) + f32 copy of the PSUM L row; bank h is released
                #    after its two reads (~1.4us), so the next slice's first
                #    PV barely waits
                zpair = znp.tile([128, SW], BF16, tag="zp", name="zpair")
                znb = znp.tile([HD, SW], BF16, tag="zb", name="znb")
                nc.vector.tensor_copy(zpair[0:HD, :], zaug[0][0:HD, :])
                lra = rrp.tile([1, SW], F32, tag="rr", name="lrow")
                nc.vector.tensor_copy(lra[0:1, :], zaug[0][HD : HD + 1, :])
                nc.vector.tensor_copy(znb[:, :], zaug[1][0:HD, :])
                lrb = rrp.tile([1, SW], F32, tag="rr", name="lrow")
                nc.vector.tensor_copy(lrb[0:1, :], zaug[1][HD : HD + 1, :])
                # 2) ONE DRAM round trip per head spreads L into [128,4]
                #    q-tile-major form (sync queue; its ~2us data stall only
                #    delays output-tile writes, which nothing waits on);
                #    head-B partition shift on gpsimd
                lis = []
                for h, lr in ((0, lra), (1, lrb)):
                    rd = drp.tile([1, SW], F32, tag=f"rd{h}", name="rd")
                    nc.sync.dma_start(out=rd[:, :], in_=lr[0:1, :])
                    li = rrp.tile([128, SW // 128], F32, tag=f"li{h}", name="li")
                    nc.sync.dma_start(
                        out=li[:, :],
                        in_=rd[0, :].rearrange("(a p) -> p a", p=128),
                    )
                    lis.append(li)
                nc.gpsimd.dma_start(out=zpair[HD:128, :], in_=znb[:, :])
                st = {"lis": lis, "ris": None}

                def part2(st=st):
                    # reciprocals: EMITTED only once the li spreads have
                    # certainly landed, so the DVE queue never blocks
                    st["ris"] = []
                    for h in range(2):
                        ri = rrp.tile(
                            [128, SW // 128], F32, tag=f"ri{h}", name="ri"
                        )
                        nc.vector.reciprocal(ri[:, :], st["lis"][h][:, :])
                        st["ris"].append(ri)

                deferred.append((g0 + 7, part2))
                for j in range(4):
                    oproj_work.append((zpair, st, sp * SW, j, g0 + 9 + 2 * j))

            def emit_oproj_qtile(item, scalar_assist=False):
                zpair, st, qs_t, j, _ = item
                assert st["ris"] is not None, "oproj before its deferred recips"
                ra, rb = st["ris"]
                jq = slice(j * 128, (j + 1) * 128)
                # two concurrent row-tiled matmuls (unnormalised zpair) into
                # two 1-bank ex tiles; evacs scale by 1/L per output row
                opa = ext.tile([128, SW], F32, tag="ex", name="ps_oa")
                nc.tensor.matmul(
                    opa[:, :],
                    lhsT=zpair[0:HD, jq],
                    rhs=wo_sb[0:HD, :],
                    start=True,
                    stop=True,
                    skip_group_check=True,
                    tile_position=(0, 0),
                )
                opb = ext.tile([128, SW], F32, tag="ex", name="ps_ob")
                nc.tensor.matmul(
                    opb[:, :],
                    lhsT=zpair[HD:128, jq],
                    rhs=wo_sb[HD:128, :],
                    start=True,
                    stop=True,
                    skip_group_check=True,
                    tile_position=(HD, 0),
                )
                o_tmp = outp.tile([128, D], F32, tag="otmp", name="o_tmp")
                if scalar_assist:
                    nc.scalar.mul(o_tmp[:, :], opb[:, :], rb[:, j : j + 1])
                else:
                    nc.vector.tensor_scalar_mul(
                        o_tmp[:, :], opb[:, :], rb[:, j : j + 1]
                    )
                o_sb = outp.tile([128, D], BF16, tag="ot", name="o_sb")
                nc.vector.scalar_tensor_tensor(
                    o_sb[:, :],
                    opa[:, :],
                    ra[:, j : j + 1],
                    o_tmp[:, :],
                    op0=mybir.AluOpType.mult,
                    op1=mybir.AluOpType.add,
                )
                r0 = qs_t + j * 128
                nc.sync.dma_start(out=out[r0 : r0 + 128, :], in_=o_sb[:, :])

            # HAM warm-up bridging preamble -> first projection
            warm_ps = ext.tile([128, SW], F32, tag="ex", name="ps_warm")
            for _ in range(20):
                nc.tensor.matmul(
                    warm_ps[:, 0:128],
                    lhsT=ident_sb[:, :],
                    rhs=mask_sb[:, :],
                    start=True,
                    stop=True,
                    skip_group_check=True,
                )

            # ---------- global unit stream ----------
            stream = []
            first_of_slice = {}
            for s in range(NS):
                qs = s * SW
                first_of_slice[s] = len(stream)
                for kb in range(4 * (s + 1)):
                    qlo = max(qs, kb * 128)
                    stream.append([s, kb, qs + SW - qlo, qlo, None])
            G = len(stream)

            def proj_items(sn):
                return [
                    lambda sn=sn: emit_proj_q(sn),
                    lambda sn=sn: emit_proj_k(sn),
                    lambda sn=sn: emit_proj_v_tt(sn, 0),
                    lambda sn=sn: emit_proj_v_tt(sn, 1),
                    lambda sn=sn: emit_proj_v_tt(sn, 2),
                    lambda sn=sn: emit_proj_v_tt(sn, 3),
                ]

            pending_proj = []  # for the upcoming slice

            def inserts(s, i, g, L):
                # 1) projection items for slice s+1 (hard deadline: all
                #    emitted before slice s+1 begins); 2/iter for short
                #    slices, else 1/iter starting at i=1
                budget = 2 if L <= 4 else 1
                did = 0
                while pending_proj and did < budget:
                    left = L - i
                    if len(pending_proj) >= left or i >= 1:
                        pending_proj.pop(0)()
                        did += 1
                    else:
                        break
                # 2) O-proj q-tiles when pacing allows and no proj emitted
                if did == 0 and oproj_work and g >= oproj_work[0][4]:
                    emit_oproj_qtile(oproj_work.pop(0))

            # prefill: slice-0 Q/K then the first scores (so exp starts
            # ASAP), then slice-0 V while exp(u0) runs
            emit_proj_q(0)
            emit_proj_k(0)
            emit_scores(stream[0])
            for tt in range(4):
                emit_proj_v_tt(0, tt)

            zaug = None
            prev_zaug = None
            for g in range(G):
                s, kb = stream[g][0], stream[g][1]
                L = 4 * (s + 1)
                i = g - first_of_slice[s]
                if i == 0 and s + 1 < NS:
                    pending_proj.extend(proj_items(s + 1))
                emit_exp(stream[g])
                if g + 1 < G:
                    emit_scores(stream[g + 1])
                if g >= 1:
                    ps_, pkb = stream[g - 1][0], stream[g - 1][1]
                    if pkb == 0:
                        # stream[g-1] opens slice ps_: finish the previous
                        # slice's accumulators, then claim fresh ones
                        if prev_zaug is not None:
                            emit_zfinish(ps_ - 1, prev_zaug, g0=g)
                        zaug = [
                            zps.tile([HD + 1, SW], F32, tag="za", name="zauga"),
                            zps.tile([HD + 1, SW], F32, tag="zb", name="zaugb"),
                        ]
                        prev_zaug = zaug
                    emit_pv(stream[g - 1], zaug)
                if deferred:
                    due = [f for dg, f in deferred if dg <= g]
                    deferred[:] = [(dg, f) for dg, f in deferred if dg > g]
                    for f in due:
                        f()
                inserts(s, i, g, L)
            emit_pv(stream[G - 1], zaug)
            for _, f in deferred:
                f()
            deferred.clear()

            # ---------- tail: slice 7 ----------
            # post-norm variant: skips the 1/L broadcast chain entirely.
            # Unnormalised zpair (stacked) feeds two CONCURRENT row-tiled
            # O-proj matmuls per q-tile into one score-pool tile's two
            # banks; the evacs scale by 1/L per output row, split across
            # the now-idle ScalarE and the DVE.
            zpair = znp.tile([128, SW], BF16, tag="zp", name="zpair_t")
            znb = znp.tile([HD, SW], BF16, tag=f"zraw1", name="znb_t")
            nc.vector.tensor_copy(zpair[0:HD, :], zaug[0][0:HD, :])
            nc.vector.tensor_copy(znb[:, :], zaug[1][0:HD, :])
            nc.gpsimd.dma_start(out=zpair[HD:128, :], in_=znb[:, :])
            ris = []
            for h in range(2):
                lr = rrp.tile([1, SW], F32, tag="rr", name="lrow_t")
                nc.vector.tensor_copy(lr[0:1, :], zaug[h][HD : HD + 1, :])
                rd = drp.tile([1, SW], F32, tag=f"rd{h}", name="rd_t")
                nc.sync.dma_start(out=rd[:, :], in_=lr[0:1, :])
                li = rrp.tile([128, SW // 128], F32, tag=f"li{h}", name="li_t")
                nc.sync.dma_start(
                    out=li[:, :], in_=rd[0, :].rearrange("(a p) -> p a", p=128)
                )
                ri = rrp.tile([128, SW // 128], F32, tag=f"ri{h}", name="ri_t")
                nc.vector.reciprocal(ri[:, :], li[:, :])
                ris.append(ri)
            for j in range(4):
                jq = slice(j * 128, (j + 1) * 128)
                op = ps.tile([128, 2 * SW], F32, tag="sg", name="ps_ot")
                nc.tensor.matmul(
                    op[:, 0:SW],
                    lhsT=zpair[0:HD, jq],
                    rhs=wo_sb[0:HD, :],
                    start=True,
                    stop=True,
                    skip_group_check=True,
                    tile_position=(0, 0),
                )
                nc.tensor.matmul(
                    op[:, SW : 2 * SW],
                    lhsT=zpair[HD:128, jq],
                    rhs=wo_sb[HD:128, :],
                    start=True,
                    stop=True,
                    skip_group_check=True,
                    tile_position=(HD, 0),
                )
                o_tmp = outp.tile([128, D], F32, tag="otmp", name="o_tmp")
                nc.scalar.mul(o_tmp[:, :], op[:, SW : 2 * SW], ris[1][:, j : j + 1])
                o_sb = outp.tile([128, D], BF16, tag="ot", name="o_sbt")
                nc.vector.scalar_tensor_tensor(
                    o_sb[:, :],
                    op[:, 0:SW],
                    ris[0][:, j : j + 1],
                    o_tmp[:, :],
                    op0=mybir.AluOpType.mult,
                    op1=mybir.AluOpType.add,
                )
                r0 = (NS - 1) * SW + j * 128
                nc.sync.dma_start(out=out[r0 : r0 + 128, :], in_=o_sb[:, :])
            # drain any remaining O-proj backlog (slice 6)
            k = 0
            while oproj_work:
                emit_oproj_qtile(oproj_work.pop(0), scalar_assist=(k % 2 == 0))
                k += 1

    _split_waits(nc)
    return nc


_NC_CACHE = {}


def _get_nc():
    if "nc" not in _NC_CACHE:
        _NC_CACHE["nc"] = build_nc()
    return _NC_CACHE["nc"]


def make_in_maps(combined_embed, W_K, b_K, W_Q, b_Q, W_V, b_V, W_O, b_O):
    f32 = np.float32
    in_maps = []
    for c in range(8):
        b = c // 4
        g = c % 4
        sl = slice(g * 128, (g + 1) * 128)
        xt = np.ascontiguousarray(np.asarray(combined_embed[b], f32).T)
        in_maps.append(
            {
                "xt": xt.astype(_BF16),
                "wq": np.ascontiguousarray(np.asarray(W_Q, f32)[:, sl]).astype(_BF16),
                "wk": np.ascontiguousarray(np.asarray(W_K, f32)[:, sl]).astype(_BF16),
                "wv": np.ascontiguousarray(np.asarray(W_V, f32)[:, sl]).astype(_BF16),
                "wo": np.ascontiguousarray(np.asarray(W_O, f32)[sl, :]).astype(_BF16),
                "bq": np.asarray(b_Q, f32)[sl].reshape(128, 1).copy(),
                "bk": np.asarray(b_K, f32)[sl].reshape(128, 1).copy(),
                "bv": np.asarray(b_V, f32)[sl].reshape(1, 128).copy(),
            }
        )
    return in_maps


def run_cores(in_maps, **kwargs):
    nc = _get_nc()
    return run_bass_kernel_spmd(nc, in_maps, core_ids=list(range(8)), **kwargs)


def kernel(
    combined_embed, W_K, b_K, W_Q, b_Q, W_V, b_V, W_O, b_O
):  # full inputs -> full output
    in_maps = make_in_maps(
        combined_embed, W_K, b_K, W_Q, b_Q, W_V, b_V, W_O, b_O
    )
    res = run_cores(in_maps)
    out = np.zeros((B, T, D), np.float32)
    for c in range(8):
        out[c // 4] += np.asarray(res.results[c]["out"], np.float32)
    out += np.asarray(b_O, np.float32)[None, None, :]
    return out


# revision 37
# speedup vs baseline: 1.2881x; 1.0004x over previous
"""Multi-head causal self-attention on 8 TRN2 NeuronCores.

Problem: B=2, T=4096, D=512, H=8 heads (hd=64), fp32 in/out.

Sharding: core c in 0..7 handles batch b = c//4 and head pair g = c%4
(heads 2g, 2g+1 -> D-slice [128g, 128g+128)). Each core computes
    partial_out = concat_h( softmax(causal(Q_h K_h^T / 8)) V_h ) @ W_O[slice]
for its two heads; the host sums the 4 partials per batch and adds b_O.

Pipeline design (v5). ScalarE exp() is the hard floor: 144 score-slot
ACTIVATEs totalling ~156us busy at (N+352)cyc/1.2GHz. Everything else
is organised so the exp stream starts early and never starves:

  - Score slot = ONE key block x BOTH heads in one [128,1024] PSUM tile
    (head A bank 0, head B bank 1), one exp ACTIVATE per slot through a
    [128,2,n] strided AP -> 144 calls.  Global unit stream: scores run
    1 unit ahead of exp, PV 1 unit behind, across slice boundaries.
  - zfinish(s): the zaug PSUM banks are released FAST by a bf16 CAST of
    the Z rows + an f32 copy of the L row per head (~2.8us DVE), so the
    next slice's first PV barely waits.  Normalisation happens off the
    critical path ON THE GPSIMD DMA QUEUE (its mid-chain DVE waits must
    never stall the sync queue or the PE): L bounces through DRAM to
    [128,4] (where DVE reciprocal is cheap), 1/L bounces back to
    q-order and is broadcast across partitions with a 0-stride DRAM
    read; fused tensor_mul then builds a pre-normalised [128,512] bf16
    zpair (head B partition-shifted by a gpsimd DMA).  The O-projection
    is then a SINGLE 128-contraction matmul per q-tile (both heads
    summed by the PE) plus one CAST evac.  O-proj items are paced
    ~9 units past the boundary so their PE matmul never enqueues
    before zpair is ready (a stalled PE insert blocks later scores).
  - Tail (slice 7): post-norm variant skipping the broadcast chain:
    unnormalised stacked zpair feeds two CONCURRENT row-tiled O-proj
    matmuls per q-tile; evacs scale by 1/L ([128,4] spread) per output
    row, split across the now-idle ScalarE and the DVE.
  - DMA plan: per-chunk 2D x^T descriptors (batched 3D gathers run
    2-5x slower).  wq rides the Scalar-engine HWDGE queue (idle before
    the first ACTIVATE) so ACT_TABLE_LOAD lands early;
    ident/mask/biases/wk/x0-1/wv/wo/x4-7 + output tiles on the sync
    queue; x2-3 + bvrep + zfinish chains on the GpSimd SWDGE queue.
    Slice-0 projections are emitted before the stream (the PE is
    otherwise idle while DMAs land); projections for slice s+1 are
    spread one small item per stream iteration.
  - Masked scores use NEG=-640 (exp(-80)~1e-35): mathematically zero
    but safe for a future int16 fast-exp path.
"""

import numpy as np

import concourse.bass as bass
import concourse.mybir as mybir
from concourse.tile import TileContext
from concourse.bass_utils import run_bass_kernel_spmd

try:
    import ml_dtypes

    _BF16 = ml_dtypes.bfloat16
except ImportError:  # pragma: no cover
    _BF16 = None

F32 = mybir.dt.float32
BF16 = mybir.dt.bfloat16

B, T, D, H = 2, 4096, 512, 8
HD = D // H  # 64
SW = 512  # q-slice width
NS = T // SW  # 8 q-slices
NKC = D // 128  # 4 contraction chunks for the projections
NTT = T // 128  # 32 t-tiles / key blocks
NEG = -640.0  # masked score offset: exp((s+NEG)/8) <= exp(-74) ~ 0


def _split_waits(nc, max_waits=1):
    """The staged walrus rejects >1 semaphore wait per instruction; hoist
    extras onto same-engine NoOps inserted right before the instruction."""
    counter = 0
    for f in nc.m.functions:
        for blk in f.blocks:
            insts = blk.instructions
            out, changed = [], False
            for ins in insts:
                si = getattr(ins, "sync_info", None)
                waits = list(si.on_wait) if si is not None and si.on_wait else []
                if len(waits) > max_waits:
                    changed = True
                    for w in waits[:-max_waits]:
                        counter += 1
                        nop = mybir.InstNoOp(
                            name=f"I-wsplit-{counter}",
                            engine=ins.engine,
                            ins=[],
                            outs=[],
                        )
                        nop.sync_info = mybir.SyncInfo(on_wait=[w], on_update=[])
                        out.append(nop)
                    ins.sync_info = mybir.SyncInfo(
                        on_wait=waits[-max_waits:], on_update=list(si.on_update)
                    )
                out.append(ins)
            if changed:
                blk.instructions = out
    return counter


def _bcast_partitions(ap, n):
    """Replace the partition dim of an AP with an n-wide 0-stride dim."""
    return bass.AP(
        tensor=ap.tensor, offset=ap.offset, ap=[[0, n]] + list(ap.ap[1:])
    )


def build_nc():
    nc = bass.Bass("TRN2")

    xt = nc.dram_tensor("xt", [D, T], BF16, kind="ExternalInput")
    wq = nc.dram_tensor("wq", [D, 128], BF16, kind="ExternalInput")
    wk = nc.dram_tensor("wk", [D, 128], BF16, kind="ExternalInput")
    wv = nc.dram_tensor("wv", [D, 128], BF16, kind="ExternalInput")
    wo = nc.dram_tensor("wo", [128, D], BF16, kind="ExternalInput")
    bq = nc.dram_tensor("bq", [128, 1], F32, kind="ExternalInput")
    bk = nc.dram_tensor("bk", [128, 1], F32, kind="ExternalInput")
    bv = nc.dram_tensor("bv", [1, 128], F32, kind="ExternalInput")
    out = nc.dram_tensor("out", [T, D], BF16, kind="ExternalOutput")

    # maskneg[k, q'] = 0 where q' >= k else NEG  (S^T diagonal subtile mask)
    mask_np = np.where(
        np.arange(128)[None, :] >= np.arange(128)[:, None], 0.0, NEG
    ).astype(np.float32)
    ident_np = np.eye(128, dtype=np.float32)
    ident_dram = nc.inline_tensor(ident_np.astype(_BF16), name="identc")
    mask_dram = nc.inline_tensor(mask_np.astype(_BF16), name="maskc")

    with TileContext(nc) as tc:
        with (
            tc.tile_pool(name="singles", bufs=1) as singles,
            tc.tile_pool(name="ps", bufs=2, space="PSUM") as ps,
            tc.tile_pool(name="ex", bufs=2, space="PSUM") as ext,
            tc.tile_pool(name="zps", bufs=1, space="PSUM") as zps,
            tc.tile_pool(name="pt", bufs=6) as ptp,
            tc.tile_pool(name="rb", bufs=2) as rbp,
            tc.tile_pool(name="rr", bufs=4) as rrp,
            tc.tile_pool(name="zn", bufs=2) as znp,
            tc.tile_pool(name="outp", bufs=3) as outp,
            tc.tile_pool(name="drp", bufs=2, space="DRAM") as drp,
        ):
            # ---------- DMA schedule ----------
            # scalar HWDGE queue (idle until first ACTIVATE): wq only, so
            # the ACT_TABLE_LOAD that precedes the warm exp lands early.
            hp = tc.high_priority()
            hp.__enter__()
            wq_sb = singles.tile([128, NKC, 128], BF16, tag="wq")
            nc.scalar.dma_start(
                out=wq_sb[:, :, :], in_=wq[:, :].rearrange("(c p) n -> p c n", p=128)
            )
            # sync queue: ident, mask, biases, wk, x0, wv, x1, x2, wo
            ident_sb = singles.tile([128, 128], BF16, tag="ident")
            mask_sb = singles.tile([128, 128], BF16, tag="mask")
            nc.sync.dma_start(out=ident_sb[:, :], in_=ident_dram[:, :])
            nc.sync.dma_start(out=mask_sb[:, :], in_=mask_dram[:, :])
            # per-chunk 2D x^T descriptors (a batched 3D gather runs 2-5x
            # slower on the DMA engine); x0 FIRST -- the Q projection (and
            # with it the whole exp stream) starts the moment x0 lands
            xts = [
                singles.tile([128, NKC, SW], BF16, tag=f"xts{s}", name=f"xts{s}")
                for s in range(NS)
            ]

            def load_xt(eng, s):
                for c in range(NKC):
                    eng.dma_start(
                        out=xts[s][:, c, :],
                        in_=xt[c * 128 : (c + 1) * 128, s * SW : (s + 1) * SW],
                    )

            load_xt(nc.sync, 0)
            wk_sb = singles.tile([128, NKC, 128], BF16, tag="wk")
            nc.sync.dma_start(
                out=wk_sb[:, :, :], in_=wk[:, :].rearrange("(c p) n -> p c n", p=128)
            )
            bq_sb = singles.tile([128, 1], F32, tag="bq")
            bk_sb = singles.tile([128, 1], F32, tag="bk")
            nc.sync.dma_start(out=bq_sb[:, :], in_=bq[:, :])
            nc.sync.dma_start(out=bk_sb[:, :], in_=bk[:, :])
            wv_sb = singles.tile([128, NKC, 128], BF16, tag="wv")
            nc.sync.dma_start(
                out=wv_sb[:, :, :], in_=wv[:, :].rearrange("(c p) n -> p c n", p=128)
            )
            hp.__exit__(None, None, None)
            load_xt(nc.sync, 1)
            wo_sb = singles.tile([128, D], BF16, tag="wo")
            nc.sync.dma_start(out=wo_sb[:, :], in_=wo[:, :])
            # gpsimd SWDGE queue: x2, x3, bvrep (the per-slice zfinish DMA
            # chains join this queue later -- their mid-chain DVE waits must
            # not block the sync queue, which carries the output tiles)
            load_xt(nc.gpsimd, 2)
            load_xt(nc.gpsimd, 3)
            bvrep_sb = singles.tile([128, 128], F32, tag="bvrep")
            nc.gpsimd.dma_start(out=bvrep_sb[:, :], in_=_bcast_partitions(bv[:, :], 128))
            for s in range(4, NS):
                load_xt(nc.sync, s)

            # preload the exp table set while DMAs stream in (the
            # ACT_TABLE_LOAD pseudo-inst lands before this in queue order)
            warm_sb = singles.tile([1, 1], BF16, tag="warm")
            nc.scalar.activation(
                out=warm_sb[:, :],
                in_=bq_sb[0:1, 0:1],
                func=mybir.ActivationFunctionType.Exp,
                scale=0.125,
            )

            qt_sb = [
                singles.tile([128, SW], BF16, tag=f"qt{s}", name=f"qt_sb{s}")
                for s in range(NS)
            ]
            kt_sb = [
                singles.tile([128, SW], BF16, tag=f"kt{s}", name=f"kt_sb{s}")
                for s in range(NS)
            ]
            # V_aug pair per key block: [128(t), 130]; cols 0:64 head A,
            # col 64 ones(A), cols 65:129 head B, col 129 ones(B)
            va_sb = [
                singles.tile([128, 2 * (HD + 1)], BF16, tag=f"va{t}", name=f"va_sb{t}")
                for t in range(NTT)
            ]
            for t in range(NTT):
                nc.vector.memset(va_sb[t][:, HD : HD + 1], 1.0)
                nc.vector.memset(va_sb[t][:, 2 * HD + 1 : 2 * HD + 2], 1.0)

            hrows = (slice(0, HD), slice(HD, 128))

            # ---------- emit helpers ----------
            def emit_proj_q(s, w_sb=None, b_sb=None, dst=None):
                w_sb = w_sb if w_sb is not None else wq_sb
                sg = ext.tile([128, SW], F32, tag="ex", name="ps_q")
                for c in range(NKC):
                    nc.tensor.matmul(
                        sg[:, :],
                        lhsT=w_sb[:, c, :],
                        rhs=xts[s][:, c, :],
                        start=(c == 0),
                        stop=(c == NKC - 1),
                        skip_group_check=True,
                    )
                nc.vector.tensor_scalar_add(
                    (dst if dst is not None else qt_sb[s])[:, :],
                    sg[:, :],
                    (b_sb if b_sb is not None else bq_sb)[:, :],
                )

            def emit_proj_k(s):
                emit_proj_q(s, w_sb=wk_sb, b_sb=bk_sb, dst=kt_sb[s])

            def emit_proj_v_tt(s, tt):
                # one ex tile (1 bank) per t-tile so the DVE evac of tile N
                # never reads a bank the PE is still writing (fatal)
                t = 4 * s + tt
                sg = ext.tile([128, SW], F32, tag="ex", name="ps_v")
                for c in range(NKC):
                    nc.tensor.matmul(
                        sg[:, 0:128],
                        lhsT=xts[s][:, c, tt * 128 : (tt + 1) * 128],
                        rhs=wv_sb[:, c, :],
                        start=(c == 0),
                        stop=(c == NKC - 1),
                        skip_group_check=True,
                    )
                # evac + b_V add in one op: dst [128,2,64] strided
                dst3 = va_sb[t][:, 0 : 2 * (HD + 1)].rearrange(
                    "p (a b) -> p a b", a=2
                )[:, :, 0:HD]
                src3 = sg[:, 0:128].rearrange("p (a b) -> p a b", a=2)
                bv3 = bvrep_sb[:, :].rearrange("p (a b) -> p a b", a=2)
                nc.vector.tensor_add(dst3, src3, bv3)

            def emit_scores(unit):
                s, kb, n, qlo = unit[:4]
                qs = s * SW
                diag = kb * 128 >= qs
                sg = ps.tile([128, 2 * SW], F32, tag="sg", name="ps_sg")
                unit[4] = sg
                for h in range(2):
                    off = h * SW
                    nc.tensor.matmul(
                        sg[:, off : off + n],
                        lhsT=kt_sb[kb // 4][
                            hrows[h], (kb % 4) * 128 : (kb % 4 + 1) * 128
                        ],
                        rhs=qt_sb[s][hrows[h], qlo - qs : qlo - qs + n],
                        start=True,
                        stop=not diag,
                        skip_group_check=True,
                        tile_position=(h * HD, 0),
                    )
                if diag:
                    for h in range(2):
                        nc.tensor.matmul(
                            sg[:, h * SW : h * SW + 128],
                            lhsT=ident_sb[:, :],
                            rhs=mask_sb[:, :],
                            start=False,
                            stop=True,
                            skip_group_check=True,
                        )

            def emit_exp(unit):
                s, kb, n, qlo, sg = unit[:5]
                pt = ptp.tile([128, 2 * SW], BF16, tag="pt", name="pt")
                in3 = sg[:, :].rearrange("p (a b) -> p a b", a=2)[:, :, 0:n]
                out3 = pt[:, 0 : 2 * n].rearrange("p (a b) -> p a b", a=2)
                nc.scalar.activation(
                    out=out3,
                    in_=in3,
                    func=mybir.ActivationFunctionType.Exp,
                    scale=0.125,
                )
                unit[4] = pt

            # int16-Schraudolph fast exp on the DVE: i16 = round(A*score+B)
            # bitcast to bf16 IS exp(score/8) to within ~3.5%; softmax
            # normalisation cancels most of it.  Masked entries (score-640)
            # land at i16 ~ [374,2589] -> tiny positive bf16 ~ 0, no
            # saturation dependence.  One DVE op replaces a 1.1us ACTIVATE
            # on the bottleneck ScalarE for ~1/7 of the units.
            SCH_A = 23.08312065  # (2^7/ln2) * 0.125
            SCH_B = 16250.0  # 127*2^7 - 6

            def emit_exp_fast(unit):
                s, kb, n, qlo, sg = unit[:5]
                pt = ptp.tile([128, 2 * SW], BF16, tag="pt", name="pt")
                in3 = sg[:, :].rearrange("p (a b) -> p a b", a=2)[:, :, 0:n]
                out3 = (
                    pt[:, 0 : 2 * n]
                    .rearrange("p (a b) -> p a b", a=2)
                    .bitcast(mybir.dt.int16)
                )
                nc.vector.tensor_scalar(
                    out3,
                    in3,
                    SCH_A,
                    SCH_B,
                    op0=mybir.AluOpType.mult,
                    op1=mybir.AluOpType.add,
                )
                unit[4] = pt

            def emit_pv(unit, zaug):
                s, kb, n, qlo, pt = unit[:5]
                qs = s * SW
                nkb = 4 * (s + 1)
                for h in range(2):
                    nc.tensor.matmul(
                        zaug[h][0 : HD + 1, qlo - qs : SW],
                        lhsT=va_sb[kb][:, h * (HD + 1) : (h + 1) * (HD + 1)],
                        rhs=pt[:, h * n : h * n + n],
                        start=(kb == 0),
                        stop=(kb == nkb - 1),
                        skip_group_check=True,
                    )

            # ---------- zfinish: fast PSUM release + pre-normalised zpair ----
            # oproj work items: (zpair, qs, j, ready_at); deferred DVE ops
            # as (due_g, fn) -- a DVE op emitted while its DMA inputs are
            # still in flight blocks the whole in-order DVE queue, which
            # blocks ex-pool WAR evacs, which stalls the PE and starves exp.
            # So every chain-dependent DVE op is EMITTED only when its data
            # has certainly landed.
            oproj_work = []
            deferred = []

            def emit_zfinish(sp, zaug, g0=0):
                # 1) per-head: bf16 CAST of the Z rows (head A straight into
                #    zpair) + f32 copy of the PSUM L row; bank h is released
                #    after its two reads (~1.4us), so the next slice's first
                #    PV barely waits
                zpair = znp.tile([128, SW], BF16, tag="zp", name="zpair")
                znb = znp.tile([HD, SW], BF16, tag="zb", name="znb")
                nc.vector.tensor_copy(zpair[0:HD, :], zaug[0][0:HD, :])
                lra = rrp.tile([1, SW], F32, tag="rr", name="lrow")
                nc.vector.tensor_copy(lra[0:1, :], zaug[0][HD : HD + 1, :])
                nc.vector.tensor_copy(znb[:, :], zaug[1][0:HD, :])
                lrb = rrp.tile([1, SW], F32, tag="rr", name="lrow")
                nc.vector.tensor_copy(lrb[0:1, :], zaug[1][HD : HD + 1, :])
                # 2) ONE DRAM round trip per head spreads L into [128,4]
                #    q-tile-major form (sync queue; its ~2us data stall only
                #    delays output-tile writes, which nothing waits on);
                #    head-B partition shift on gpsimd
                lis = []
                for h, lr in ((0, lra), (1, lrb)):
                    rd = drp.tile([1, SW], F32, tag=f"rd{h}", name="rd")
                    nc.sync.dma_start(out=rd[:, :], in_=lr[0:1, :])
                    li = rrp.tile([128, SW // 128], F32, tag=f"li{h}", name="li")
                    nc.sync.dma_start(
                        out=li[:, :],
                        in_=rd[0, :].rearrange("(a p) -> p a", p=128),
                    )
                    lis.append(li)
                nc.gpsimd.dma_start(out=zpair[HD:128, :], in_=znb[:, :])
                st = {"lis": lis, "ris": None}

                def part2(st=st):
                    # reciprocals: EMITTED only once the li spreads have
                    # certainly landed, so the DVE queue never blocks
                    st["ris"] = []
                    for h in range(2):
                        ri = rrp.tile(
                            [128, SW // 128], F32, tag=f"ri{h}", name="ri"
                        )
                        nc.vector.reciprocal(ri[:, :], st["lis"][h][:, :])
                        st["ris"].append(ri)

                deferred.append((g0 + 7, part2))
                for j in range(4):
                    oproj_work.append((zpair, st, sp * SW, j, g0 + 9 + 2 * j))

            def emit_oproj_qtile(item, scalar_assist=False):
                zpair, st, qs_t, j, _ = item
                assert st["ris"] is not None, "oproj before its deferred recips"
                ra, rb = st["ris"]
                jq = slice(j * 128, (j + 1) * 128)
                # two concurrent row-tiled matmuls (unnormalised zpair) into
                # two 1-bank ex tiles; evacs scale by 1/L per output row
                opa = ext.tile([128, SW], F32, tag="ex", name="ps_oa")
                nc.tensor.matmul(
                    opa[:, :],
                    lhsT=zpair[0:HD, jq],
                    rhs=wo_sb[0:HD, :],
                    start=True,
                    stop=True,
                    skip_group_check=True,
                    tile_position=(0, 0),
                )
                opb = ext.tile([128, SW], F32, tag="ex", name="ps_ob")
                nc.tensor.matmul(
                    opb[:, :],
                    lhsT=zpair[HD:128, jq],
                    rhs=wo_sb[HD:128, :],
                    start=True,
                    stop=True,
                    skip_group_check=True,
                    tile_position=(HD, 0),
                )
                o_tmp = outp.tile([128, D], F32, tag="otmp", name="o_tmp")
                if scalar_assist:
                    nc.scalar.mul(o_tmp[:, :], opb[:, :], rb[:, j : j + 1])
                else:
                    nc.vector.tensor_scalar_mul(
                        o_tmp[:, :], opb[:, :], rb[:, j : j + 1]
                    )
                o_sb = outp.tile([128, D], BF16, tag="ot", name="o_sb")
                nc.vector.scalar_tensor_tensor(
                    o_sb[:, :],
                    opa[:, :],
                    ra[:, j : j + 1],
                    o_tmp[:, :],
                    op0=mybir.AluOpType.mult,
                    op1=mybir.AluOpType.add,
                )
                r0 = qs_t + j * 128
                nc.sync.dma_start(out=out[r0 : r0 + 128, :], in_=o_sb[:, :])

            # HAM warm-up bridging preamble -> first projection
            warm_ps = ext.tile([128, SW], F32, tag="ex", name="ps_warm")
            for _ in range(20):
                nc.tensor.matmul(
                    warm_ps[:, 0:128],
                    lhsT=ident_sb[:, :],
                    rhs=mask_sb[:, :],
                    start=True,
                    stop=True,
                    skip_group_check=True,
                )

            # ---------- global unit stream ----------
            stream = []
            first_of_slice = {}
            for s in range(NS):
                qs = s * SW
                first_of_slice[s] = len(stream)
                for kb in range(4 * (s + 1)):
                    qlo = max(qs, kb * 128)
                    stream.append([s, kb, qs + SW - qlo, qlo, None])
            G = len(stream)

            # offload ~1/4 of eligible (mid-slice, s>=3) units to fast exp
            fast_units = set()
            fk = 0
            for g in range(G):
                fs = stream[g][0]
                fL = 4 * (fs + 1)
                fi = g - first_of_slice[fs]
                if fs >= 3 and 3 <= fi <= fL - 4:
                    fk += 1
                    if fk % 4 == 2:
                        fast_units.add(g)

            def proj_items(sn):
                return [
                    lambda sn=sn: emit_proj_q(sn),
                    lambda sn=sn: emit_proj_k(sn),
                    lambda sn=sn: emit_proj_v_tt(sn, 0),
                    lambda sn=sn: emit_proj_v_tt(sn, 1),
                    lambda sn=sn: emit_proj_v_tt(sn, 2),
                    lambda sn=sn: emit_proj_v_tt(sn, 3),
                ]

            pending_proj = []  # for the upcoming slice

            def inserts(s, i, g, L):
                # 1) projection items for slice s+1 (hard deadline: all
                #    emitted before slice s+1 begins); 2/iter for short
                #    slices, else 1/iter starting at i=1
                budget = 2 if L <= 4 else 1
                did = 0
                while pending_proj and did < budget:
                    left = L - i
                    if len(pending_proj) >= left or i >= 1:
                        pending_proj.pop(0)()
                        did += 1
                    else:
                        break
                # 2) O-proj q-tiles when pacing allows and no proj emitted
                if did == 0 and oproj_work and g >= oproj_work[0][4]:
                    emit_oproj_qtile(oproj_work.pop(0))

            # prefill: slice-0 Q/K then the first scores (so exp starts
            # ASAP), then slice-0 V while exp(u0) runs
            emit_proj_q(0)
            emit_proj_k(0)
            emit_scores(stream[0])
            for tt in range(4):
                emit_proj_v_tt(0, tt)

            zaug = None
            prev_zaug = None
            for g in range(G):
                s, kb = stream[g][0], stream[g][1]
                L = 4 * (s + 1)
                i = g - first_of_slice[s]
                if i == 0 and s + 1 < NS:
                    pending_proj.extend(proj_items(s + 1))
                if g in fast_units:
                    emit_exp_fast(stream[g])
                else:
                    emit_exp(stream[g])
                if g + 1 < G:
                    emit_scores(stream[g + 1])
                if g >= 1:
                    ps_, pkb = stream[g - 1][0], stream[g - 1][1]
                    if pkb == 0:
                        # stream[g-1] opens slice ps_: finish the previous
                        # slice's accumulators, then claim fresh ones
                        if prev_zaug is not None:
                            emit_zfinish(ps_ - 1, prev_zaug, g0=g)
                        zaug = [
                            zps.tile([HD + 1, SW], F32, tag="za", name="zauga"),
                            zps.tile([HD + 1, SW], F32, tag="zb", name="zaugb"),
                        ]
                        prev_zaug = zaug
                    emit_pv(stream[g - 1], zaug)
                if deferred:
                    due = [f for dg, f in deferred if dg <= g]
                    deferred[:] = [(dg, f) for dg, f in deferred if dg > g]
                    for f in due:
                        f()
                inserts(s, i, g, L)
            emit_pv(stream[G - 1], zaug)
            for _, f in deferred:
                f()
            deferred.clear()

            # ---------- tail: slice 7 ----------
            # post-norm variant: skips the 1/L broadcast chain entirely.
            # Unnormalised zpair (stacked) feeds two CONCURRENT row-tiled
            # O-proj matmuls per q-tile into one score-pool tile's two
            # banks; the evacs scale by 1/L per output row, split across
            # the now-idle ScalarE and the DVE.
            # L-row copies FIRST: the serial DRAM round-trip for the 1/L
            # spread is the tail's critical path
            lrs_t, rds_t = [], []
            for h in range(2):
                lr = rrp.tile([1, SW], F32, tag="rr", name="lrow_t")
                nc.vector.tensor_copy(lr[0:1, :], zaug[h][HD : HD + 1, :])
                rd = drp.tile([1, SW], F32, tag=f"rd{h}", name="rd_t")
                nc.sync.dma_start(out=rd[:, :], in_=lr[0:1, :])
                lrs_t.append(lr)
                rds_t.append(rd)
            zpair = znp.tile([128, SW], BF16, tag="zp", name="zpair_t")
            znb = znp.tile([HD, SW], BF16, tag=f"zraw1", name="znb_t")
            nc.vector.tensor_copy(zpair[0:HD, :], zaug[0][0:HD, :])
            nc.vector.tensor_copy(znb[:, :], zaug[1][0:HD, :])
            nc.gpsimd.dma_start(out=zpair[HD:128, :], in_=znb[:, :])
            ris = []
            for h in range(2):
                li = rrp.tile([128, SW // 128], F32, tag=f"li{h}", name="li_t")
                nc.sync.dma_start(
                    out=li[:, :],
                    in_=rds_t[h][0, :].rearrange("(a p) -> p a", p=128),
                )
                ri = rrp.tile([128, SW // 128], F32, tag=f"ri{h}", name="ri_t")
                nc.vector.reciprocal(ri[:, :], li[:, :])
                ris.append(ri)
            for j in range(4):
                jq = slice(j * 128, (j + 1) * 128)
                op = ps.tile([128, 2 * SW], F32, tag="sg", name="ps_ot")
                nc.tensor.matmul(
                    op[:, 0:SW],
                    lhsT=zpair[0:HD, jq],
                    rhs=wo_sb[0:HD, :],
                    start=True,
                    stop=True,
                    skip_group_check=True,
                    tile_position=(0, 0),
                )
                nc.tensor.matmul(
                    op[:, SW : 2 * SW],
                    lhsT=zpair[HD:128, jq],
                    rhs=wo_sb[HD:128, :],
                    start=True,
                    stop=True,
                    skip_group_check=True,
                    tile_position=(HD, 0),
                )
                o_tmp = outp.tile([128, D], F32, tag="otmp", name="o_tmp")
                nc.scalar.mul(o_tmp[:, :], op[:, SW : 2 * SW], ris[1][:, j : j + 1])
                o_sb = outp.tile([128, D], BF16, tag="ot", name="o_sbt")
                nc.vector.scalar_tensor_tensor(
                    o_sb[:, :],
                    op[:, 0:SW],
                    ris[0][:, j : j + 1],
                    o_tmp[:, :],
                    op0=mybir.AluOpType.mult,
                    op1=mybir.AluOpType.add,
                )
                r0 = (NS - 1) * SW + j * 128
                nc.sync.dma_start(out=out[r0 : r0 + 128, :], in_=o_sb[:, :])
            # drain any remaining O-proj backlog (slice 6)
            k = 0
            while oproj_work:
                emit_oproj_qtile(oproj_work.pop(0), scalar_assist=(k % 2 == 0))
                k += 1

    _split_waits(nc)
    return nc


_NC_CACHE = {}


def _get_nc():
    if "nc" not in _NC_CACHE:
        _NC_CACHE["nc"] = build_nc()
    return _NC_CACHE["nc"]


def make_in_maps(combined_embed, W_K, b_K, W_Q, b_Q, W_V, b_V, W_O, b_O):
    f32 = np.float32
    in_maps = []
    for c in range(8):
        b = c // 4
        g = c % 4
        sl = slice(g * 128, (g + 1) * 128)
        xt = np.ascontiguousarray(np.asarray(combined_embed[b], f32).T)
        in_maps.append(
            {
                "xt": xt.astype(_BF16),
                "wq": np.ascontiguousarray(np.asarray(W_Q, f32)[:, sl]).astype(_BF16),
                "wk": np.ascontiguousarray(np.asarray(W_K, f32)[:, sl]).astype(_BF16),
                "wv": np.ascontiguousarray(np.asarray(W_V, f32)[:, sl]).astype(_BF16),
                "wo": np.ascontiguousarray(np.asarray(W_O, f32)[sl, :]).astype(_BF16),
                "bq": np.asarray(b_Q, f32)[sl].reshape(128, 1).copy(),
                "bk": np.asarray(b_K, f32)[sl].reshape(128, 1).copy(),
                "bv": np.asarray(b_V, f32)[sl].reshape(1, 128).copy(),
            }
        )
    return in_maps


def run_cores(in_maps, **kwargs):
    nc = _get_nc()
    return run_bass_kernel_spmd(nc, in_maps, core_ids=list(range(8)), **kwargs)


def kernel(
    combined_embed, W_K, b_K, W_Q, b_Q, W_V, b_V, W_O, b_O
):  # full inputs -> full output
    in_maps = make_in_maps(
        combined_embed, W_K, b_K, W_Q, b_Q, W_V, b_V, W_O, b_O
    )
    res = run_cores(in_maps)
    out = np.zeros((B, T, D), np.float32)
    for c in range(8):
        out[c // 4] += np.asarray(res.results[c]["out"], np.float32)
    out += np.asarray(b_O, np.float32)[None, None, :]
    return out
